# revision 8
# baseline (speedup 1.0000x reference)
"""Trainium2 kernel for nn_InterpolatorMaskArgs (embedding_lookup, memory regime).

reference computes:  ind = floor((x[0]-X0)/DX);  res = sum(roll(mask, ind) * yOrig)
i.e. a full O(N) dot product between yOrig and the rolled mask, with an
out-of-range guard on x.

Strategy (matches the sharding hint):
  - 1-D shard yOrig along N across the 8 cores (contiguous 2M-element shards).
  - The roll is resolved at shard time: core c receives the slice of the
    rolled mask aligned with its yOrig shard, i.e. mask[(c*S - ind) mod N ...]
    (mod-N wraparound == the halo exchange, done while scattering inputs).
  - Both inputs are cast to fp16 on the host (tolerance is 2e-2; measured
    end-to-end rel err ~1e-3).  8 MiB/core of DMA; the binding resource is
    the SDMA SBUF-write side at ~340-360 GB/s aggregate (fp8 variants don't
    help: cast-DMA still writes fp16, and fp8-consuming DVE ops measured
    4-5x slower than fp16 2x mode).
  - THREE DMA rings run concurrently (sync HWDGE, scalar HWDGE, gpsimd
    SWDGE) so per-transfer completion overheads overlap.  The y and mask
    units of consecutive tiles are striped across the rings pairwise in tile
    order, with tile 0/1 units on the fast-starting HWDGE rings (SWDGE's Q7
    descriptor path starts ~3.5 us late), so tile readiness is monotone and
    compute starts as early as possible.
  - Both full shards live in SBUF (8 MiB of 24) so nothing gates the rings;
    VectorE chases the per-ring completion semaphores.  First and last
    tiles are tapered to 1024 wide to cut pipeline-fill and tail latency.
  - Per tile: VectorE fp16 in-place product (DVE 2x mode).  The free-dim
    reduction to fp32 partials is split: most tiles on the otherwise idle
    ScalarE (activation-Copy accum_out), two mid-stream tiles on VectorE
    (tensor_scalar accum) so neither engine exceeds the DMA stream time.
    (The fused tensor_tensor_reduce would do mul+reduce in one op but this
    walrus build rejects its ISA encoding.)
  - The final all-reduce of the 8*128*NTILES fp32 partials is done on the
    host (a few KB), followed by the out-of-range predicate.
"""

import numpy as np

import concourse.bass as bass
import concourse.mybir as mybir
from concourse.bass_utils import run_bass_kernel_spmd

# Grid constants (must match the problem's reference.py)
N = 16777216
X0 = 0.0
DX = 1.0
XMAX = X0 + (N - 1) * DX

NCORES = 8
P = 128                 # SBUF partitions
S = N // NCORES         # 2,097,152 elements per core
F = S // P              # 16,384 free-dim elements per partition

# Compute tiles (free-dim ranges): tapered 1024 at both ends, 2048 inside.
TILES = [(0, 1024), (1024, 2048),
         (2048, 4096), (4096, 6144), (6144, 8192),
         (8192, 10240), (10240, 12288), (12288, 14336),
         (14336, 15360), (15360, 16384)]
NTILES = len(TILES)
VEC_ACC = (3, 6)        # tiles reduced on VectorE; the rest on ScalarE

# DMA unit schedule: tile i's y unit and m unit striped across the 3 rings.
# Each entry: ('y'|'m', tile). Ring order == FIFO completion order.
RING_S = [('y', 0), ('y', 1), ('m', 2), ('y', 4), ('m', 5), ('y', 7), ('m', 8)]
RING_A = [('m', 0), ('m', 1), ('y', 3), ('m', 4), ('y', 6), ('m', 7), ('y', 9)]
RING_G = [('y', 2), ('m', 3), ('y', 5), ('m', 6), ('y', 8), ('m', 9)]


def _waits():
    """tile -> [(ring, count), (ring, count)] for its y and m units."""
    w = {}
    for ring_name, sched in (('s', RING_S), ('a', RING_A), ('g', RING_G)):
        for pos, (kind, tile) in enumerate(sched):
            w.setdefault(tile, []).append((ring_name, 16 * (pos + 1)))
    return w


_TILE_WAITS = _waits()
_CACHED_NC = None


def _build_nc():
    """Raw Bass (not Tile): this walrus build rejects instructions carrying
    more than ~1 inline semaphore wait ("Too many sync wait commands"), so
    all cross-engine sync uses standalone wait_ge instructions."""
    nc = bass.Bass(trn_type="TRN2")
    f16 = mybir.dt.float16
    f32 = mybir.dt.float32
    yt = nc.dram_tensor("yt", [P, F], f16, kind="ExternalInput")
    mt = nc.dram_tensor("mt", [P, F], f16, kind="ExternalInput")
    out = nc.dram_tensor("out", [P, NTILES], f32, kind="ExternalOutput")

    with (
        nc.Block() as block,
        nc.semaphore("ds") as ds,
        nc.semaphore("da") as da,
        nc.semaphore("dg") as dg,
        nc.semaphore("mul_sem") as mul_sem,
        nc.semaphore("acc_sem") as acc_sem,
        nc.semaphore("out_sem") as out_sem,
        nc.sbuf_tensor("ys", [P, F], f16) as ys,
        nc.sbuf_tensor("ms", [P, F], f16) as ms,
        nc.sbuf_tensor("acc", [P, NTILES], f32) as acc,
    ):
        sems = {'s': ds, 'a': da, 'g': dg}

        def issue_units(eng, sched, sem):
            for kind, tile in sched:
                a, b = TILES[tile]
                src = yt if kind == 'y' else mt
                dst = ys if kind == 'y' else ms
                eng.dma_start(out=dst[:, a:b], in_=src[:, a:b]).then_inc(sem, 16)

        @block.sync
        def _(sync):
            issue_units(sync, RING_S, ds)
            sync.wait_ge(acc_sem, NTILES)
            sync.dma_start(out=out[:], in_=acc[:]).then_inc(out_sem, 16)
            sync.wait_ge(out_sem, 16)

        @block.gpsimd
        def _(gpsimd):
            issue_units(gpsimd, RING_G, dg)

        @block.vector
        def _(vector):
            for i, (a, b) in enumerate(TILES):
                for ring_name, count in _TILE_WAITS[i]:
                    vector.wait_ge(sems[ring_name], count)
                # in-place product into the y shard (fp16 -> DVE 2x mode)
                nc.vector.tensor_mul(
                    out=ys[:, a:b], in0=ys[:, a:b], in1=ms[:, a:b]
                ).then_inc(mul_sem, 1)
                if i in VEC_ACC:
                    nc.vector.tensor_scalar(
                        ys[:, a:b], ys[:, a:b], 1.0, 0.0,
                        op0=mybir.AluOpType.mult, op1=mybir.AluOpType.add,
                        accum_out=acc[:, i:i + 1],
                    ).then_inc(acc_sem, 1)

        @block.scalar
        def _(scalar):
            # issue this ring's DMA units up front on the ACT sequencer
            issue_units(scalar, RING_A, da)
            for i, (a, b) in enumerate(TILES):
                if i in VEC_ACC:
                    continue
                scalar.wait_ge(mul_sem, i + 1)
                # acc[:, i] = per-partition free-dim sum of the product;
                # the mandatory full-width copy lands in the (dead) mask tile
                nc.scalar.activation(
                    out=ms[:, a:b],
                    in_=ys[:, a:b],
                    func=mybir.ActivationFunctionType.Copy,
                    accum_out=acc[:, i:i + 1],
                ).then_inc(acc_sem, 1)

    return nc


def _get_nc():
    global _CACHED_NC
    if _CACHED_NC is None:
        _CACHED_NC = _build_nc()
    return _CACHED_NC


def kernel(x, yOrig, mask):
    x = np.asarray(x)
    yOrig = np.asarray(yOrig, dtype=np.float32)
    mask = np.asarray(mask, dtype=np.float32)

    xs = float(x.reshape(-1)[0])
    ind = int(np.floor((xs - X0) / DX))
    shift = ind % N

    # rolled[i] = mask[(i - ind) mod N]  (== np.roll(mask, ind))
    if shift == 0:
        rolled = mask
    else:
        rolled = np.concatenate([mask[N - shift:], mask[:N - shift]])

    yq = yOrig.astype(np.float16)
    mq = rolled.astype(np.float16)

    in_maps = []
    for c in range(NCORES):
        in_maps.append({
            "yt": yq[c * S:(c + 1) * S].reshape(P, F),
            "mt": mq[c * S:(c + 1) * S].reshape(P, F),
        })

    res = run_bass_kernel_spmd(_get_nc(), in_maps, core_ids=list(range(NCORES)))

    partials = np.concatenate([r["out"].reshape(-1) for r in res.results])
    total = np.float32(partials.sum(dtype=np.float32))

    if xs >= XMAX or xs < X0:
        total = np.float32(0.0)

    # Stash for test harnesses that want profiling info.
    kernel.last_results = res
    return np.asarray(total, dtype=np.float32)


# revision 10
# speedup vs baseline: 1.2514x; 1.2514x over previous
"""v10b: v7c rings + early mid-stream TS (2,4) + vector-reduced 512 tail tiles.

- fp16 y and mask packed per tile: one DMA per tile brings both halves
  (rows of 4*T_k bytes -> 8 KiB descriptors for the 2048-wide bulk tiles).
- Tiles alternate between the sync and scalar HWDGE rings so consecutive
  transfers overlap their ~1.6 us completion-receipt overheads.  No SWDGE
  (gpsimd) ring: it starts ~3.5 us late and its Q7 descriptor traffic slows
  the compute engines.
- Full shard lives in SBUF; no buffer recycling gates the rings.
- Tail tiles taper to 1024/512 wide; the last two reduce on VectorE
  (tensor_scalar accum) right after their product, so the tail is
  ~TT+TS+out-DMA only.
"""
import numpy as np

import concourse.bass as bass
import concourse.mybir as mybir
from concourse.bass_utils import run_bass_kernel_spmd

N = 16777216
X0 = 0.0
DX = 1.0
XMAX = X0 + (N - 1) * DX

NCORES = 8
P = 128
S = N // NCORES
F = S // P

T_K = [2048] * 7 + [1024, 512, 512]      # tile widths, sum = F
NTILES = len(T_K)
STARTS = np.cumsum([0] + T_K).tolist()    # in the logical F dimension
OFFS = [2 * s for s in STARTS]            # in the packed 2F-wide SBUF/DRAM
W2 = 2 * F                                # packed row width
RING_S = [0, 2, 4, 6, 8]                  # tiles on the sync ring
RING_A = [1, 3, 5, 7, 9]                  # tiles on the scalar ring
# Tiles reduced on VectorE (tensor_scalar accum).  Mid-stream ONLY: the DVE
# holds the accumulation in a cache that an auto-emitted DVE_READ_ACCUMULATOR
# flushes to SBUF after the sem-incrementing instruction — a vector-reduced
# LAST tile races the out-DMA against that flush (observed garbage columns).
VEC_ACC = (2, 4, 8, 9)
TAIL_VEC = (8, 9)   # tail TS tiles: inc deferred to the flush-guard dummy

_CACHED_NC = None


def _build_nc():
    nc = bass.Bass(trn_type="TRN2")
    f16 = mybir.dt.float16
    f32 = mybir.dt.float32
    ym = nc.dram_tensor("ym", [P, W2], f16, kind="ExternalInput")
    out = nc.dram_tensor("out", [P, NTILES], f32, kind="ExternalOutput")

    import contextlib
    with contextlib.ExitStack() as stack:
        block = stack.enter_context(nc.Block())
        # One semaphore PER TRANSFER: with several transfers pipelined on a
        # ring, a cumulative sem at 16*(pos+1) does NOT imply transfer pos
        # finished (one SDMA engine can run ahead on transfer pos+1 while
        # another lags on pos) — observed as corrupted tiles under tracing.
        tsem = [stack.enter_context(nc.semaphore(f"dt{i}")) for i in range(NTILES)]
        mul_sem = stack.enter_context(nc.semaphore("mul_sem"))
        acc_sem = stack.enter_context(nc.semaphore("acc_sem"))
        out_sem = stack.enter_context(nc.semaphore("out_sem"))
        ys = stack.enter_context(nc.sbuf_tensor("ys", [P, W2], f16))
        acc = stack.enter_context(nc.sbuf_tensor("acc", [P, NTILES], f32))
        scr = stack.enter_context(nc.sbuf_tensor("scr", [P, 2], f32))

        def issue(eng, tiles):
            for t in tiles:
                lo, hi = OFFS[t], OFFS[t + 1]
                eng.dma_start(out=ys[:, lo:hi], in_=ym[:, lo:hi]).then_inc(tsem[t], 16)

        @block.sync
        def _(sync):
            issue(sync, RING_S)
            sync.wait_ge(acc_sem, NTILES - len(TAIL_VEC) + 1)
            sync.dma_start(out=out[:], in_=acc[:]).then_inc(out_sem, 16)
            sync.wait_ge(out_sem, 16)

        @block.vector
        def _(vector):
            for i in range(NTILES):
                vector.wait_ge(tsem[i], 16)
                lo = OFFS[i]
                mid = lo + T_K[i]
                hi = OFFS[i + 1]
                nc.vector.tensor_mul(
                    out=ys[:, lo:mid], in0=ys[:, lo:mid], in1=ys[:, mid:hi]
                ).then_inc(mul_sem, 1)
                if i in VEC_ACC:
                    ts = nc.vector.tensor_scalar(
                        ys[:, lo:mid], ys[:, lo:mid], 1.0, 0.0,
                        op0=mybir.AluOpType.mult, op1=mybir.AluOpType.add,
                        accum_out=acc[:, i:i + 1],
                    )
                    if i not in TAIL_VEC:
                        ts.then_inc(acc_sem, 1)
            # flush guard: ordered after the tail TS ops' compiler-emitted
            # DVE_READ_ACCUMULATOR flushes; its inc releases the out-DMA
            nc.vector.tensor_copy(out=scr[:], in_=acc[:, 0:2]).then_inc(acc_sem, 1)

        @block.scalar
        def _(scalar):
            issue(scalar, RING_A)
            for i in range(NTILES):
                if i in VEC_ACC:
                    continue
                scalar.wait_ge(mul_sem, i + 1)
                lo = OFFS[i]
                mid = lo + T_K[i]
                hi = OFFS[i + 1]
                nc.scalar.activation(
                    out=ys[:, mid:hi],
                    in_=ys[:, lo:mid],
                    func=mybir.ActivationFunctionType.Copy,
                    accum_out=acc[:, i:i + 1],
                ).then_inc(acc_sem, 1)

    return nc


def _get_nc():
    global _CACHED_NC
    if _CACHED_NC is None:
        _CACHED_NC = _build_nc()
    return _CACHED_NC


def kernel(x, yOrig, mask):
    x = np.asarray(x)
    yOrig = np.asarray(yOrig, dtype=np.float32)
    mask = np.asarray(mask, dtype=np.float32)

    xs = float(x.reshape(-1)[0])
    ind = int(np.floor((xs - X0) / DX))
    shift = ind % N

    if shift == 0:
        rolled = mask
    else:
        rolled = np.concatenate([mask[N - shift:], mask[:N - shift]])

    yq = yOrig.astype(np.float16)
    mq = rolled.astype(np.float16)

    in_maps = []
    for c in range(NCORES):
        yr = yq[c * S:(c + 1) * S].reshape(P, F)
        mr = mq[c * S:(c + 1) * S].reshape(P, F)
        ymc = np.empty((P, W2), dtype=np.float16)
        for t in range(NTILES):
            a, b = STARTS[t], STARTS[t + 1]
            lo = OFFS[t]
            mid = lo + T_K[t]
            hi = OFFS[t + 1]
            ymc[:, lo:mid] = yr[:, a:b]
            ymc[:, mid:hi] = mr[:, a:b]
        in_maps.append({"ym": ymc})

    res = run_bass_kernel_spmd(_get_nc(), in_maps, core_ids=list(range(NCORES)))

    partials = np.concatenate([r["out"].reshape(-1) for r in res.results])
    total = np.float32(partials.sum(dtype=np.float32))

    if xs >= XMAX or xs < X0:
        total = np.float32(0.0)

    kernel.last_results = res
    return np.asarray(total, dtype=np.float32)
